# revision 13
# baseline (speedup 1.0000x reference)
"""Bahdanau attention TRN2 kernel.

Reference math (per batch b):
    qe = query @ W1 + b1                       # [Tq, U]
    ve = values @ W2 + b2                      # [Tv, U]
    score[q, v] = sum_u V[u] * tanh(qe[q, u] + ve[v, u])   (+ bV, dropped:
                  softmax over v is shift-invariant and score is not an
                  output, so bV cancels exactly)
    attn = softmax(score, axis=v)
    context = attn @ values

Sharding: 8 cores = 4 batches x 2 halves of Tq; softmax over Tv is local.

Per-core dataflow (U=256 on partitions as 2 chunks of 128):
  - Host ships layout-transformed inputs: query^T and values^T (so the
    contraction dim is on partitions -- no on-device transposes), plus a
    packed constants array (identity, ones, b1/b2, V columns, W1, W2).
  - PE projects qe_T[u, q] and ve_T[u, v]; b1+b2 folded into qe_T via a
    K=2 ones-matmul (fp32).  A few identity matmuls warm the PE clock
    (HAM) while input DMAs are in flight.
  - Broadcast-add + tanh, engine-balanced per measured rates (DVE
    tensor_scalar 263ns/op, GPSIMD tensor_tensor ~870ns/op, ACT grouped
    tanh ~230ns/q-chunk, ACT direct fused bias+tanh 491ns/op).  tanh
    output H is MM_DTYPE (bf16) for the PE reduction; score still
    accumulates in fp32 PSUM.
  - V-reduction on PE: [K=128, M=1, N=512] matmuls (2 queries each),
    accumulated over the 2 U-chunks (chunk-0 passes emitted before
    chunk-1 so independent matmuls pipeline).  tile_position col-tiling
    spreads rows over PSUM partitions {0,32,64,96} x 4 free slots of a
    2-bank tile (16 q's); a full-tile copy (ACT/DVE alternating) moves it
    to SBUF and an SBUF->SBUF DMA regathers score[q, v] rows.
  - Softmax pipelined per 32-query slice (ACT Exp + fused accum_out
    row-sum, DVE reciprocal + tensor_scalar mul); context at the end via
    PE transpose of attn and two accumulating matmuls against values.
"""

from contextlib import ExitStack

import numpy as np

import concourse.tile as tile
from concourse import bacc, mybir
from concourse.bass_utils import run_bass_kernel_spmd

F32 = mybir.dt.float32
BF16 = mybir.dt.bfloat16

N_CORES = 8
B, TQ, TV = 4, 256, 256
H, D, U = 512, 512, 256
TQS = TQ // 2          # 128 queries per core
P = 128
UC = U // P            # 2 partition chunks of U
G = 16                 # queries per tanh group
NG = TQS // G
QPT = 16               # queries per PSUM score tile (2 banks)
GPT = QPT // G         # groups per score tile

# engine-balance tunables
DIRECT_Q = 1           # q's per group routed via ACT fused bias+tanh
GPS_Q = 2              # q's per group staged by GPSIMD tensor_tensor
STAGE_ON_ACT = 2       # of every 4 psum->sbuf score copies, how many on ACT
WARMUP_MM = 10         # identity matmuls to warm the PE clock
MM_DTYPE = "bf16"      # V-reduction dtype: bf16 | f32 | f32r

# wpack column layout (host-side packing of constants)
_IDENT_O = 0
_VCOL_O = _IDENT_O + P
_W2_O = _VCOL_O + UC          # W2 early: it gates the ve projection
_ONES_O = _W2_O + 4 * U       # rows 0-1 used
_B12_O = _ONES_O + P          # rows 0-1 used: b1 | b2
_W1_O = _B12_O + U
_WPACK_COLS = _W1_O + 4 * U
_WSPLIT = _ONES_O             # DMA 1: ident..W2, DMA 2: ones..W1

_PROGRAM = None


def _build_program():
    nc = bacc.Bacc("TRN2", target_bir_lowering=False, debug=False,
                   num_devices=N_CORES)

    qt_in = nc.declare_dram_parameter("qT", [P, 4 * TQS], F32, isOutput=False)
    vt_in = nc.declare_dram_parameter("vT", [P, 4 * TV], F32, isOutput=False)
    v_in = nc.declare_dram_parameter("v", [TV, D], F32, isOutput=False)
    wp_in = nc.declare_dram_parameter("wpack", [P, _WPACK_COLS], F32,
                                      isOutput=False)
    ctx_out = nc.declare_dram_parameter("ctx", [TQS, D], F32, isOutput=True)
    attn_out = nc.declare_dram_parameter("attn", [TQS, TV], F32, isOutput=True)

    h_dt = {"bf16": BF16, "f32": F32, "f32r": F32}[MM_DTYPE]

    def mm_ap(ap):
        return ap.bitcast(mybir.dt.float32r) if MM_DTYPE == "f32r" else ap

    with tile.TileContext(nc) as tc, ExitStack() as octx:
        consts = octx.enter_context(tc.tile_pool(name="consts", bufs=1))
        work = octx.enter_context(tc.tile_pool(name="work", bufs=1))
        s_pool = octx.enter_context(tc.tile_pool(name="s", bufs=3))
        stage_pool = octx.enter_context(tc.tile_pool(name="stage", bufs=2))

        score_sb = work.tile([P, TV], F32, name="score", tag="score")

        # ---- inputs ----
        wp = consts.tile([P, _WPACK_COLS], F32, name="wp", tag="wp")
        nc.sync.dma_start(wp[:, 0:_WSPLIT], wp_in[:, 0:_WSPLIT])
        ident = wp[:, _IDENT_O:_IDENT_O + P]
        v_col = wp[:, _VCOL_O:_VCOL_O + UC]
        w2_sb = wp[:, _W2_O:_W2_O + 4 * U]

        vT = consts.tile([P, 4 * TV], F32, name="vT", tag="vT")
        nc.sync.dma_start(vT[:], vt_in[:])
        qT = consts.tile([P, 4 * TQS], F32, name="qT", tag="qT")
        nc.sync.dma_start(qT[:], qt_in[:])

        nc.sync.dma_start(wp[:, _WSPLIT:], wp_in[:, _WSPLIT:])
        ones2 = wp[0:2, _ONES_O:_ONES_O + P]
        b12 = wp[0:2, _B12_O:_B12_O + U]
        w1_sb = wp[:, _W1_O:_W1_O + 4 * U]

        values_big = consts.tile([P, 2 * D], F32, name="values", tag="values")
        nc.sync.dma_start(values_big[:].rearrange("p (a d) -> p a d", a=2),
                          v_in[:].rearrange("(a p) d -> p a d", p=P))
        values_sb = [values_big[:, a * D:(a + 1) * D] for a in range(2)]

        # throwaway matmuls warm the PE clock (HAM) during the DMA wait;
        # result parked in score_sb (fully overwritten later)
        with tc.tile_pool(name="warm_ps", bufs=1, space="PSUM") as warm_ps:
            wps = warm_ps.tile([P, P], F32, name="wps", tag="wps")
            for _ in range(WARMUP_MM):
                nc.tensor.matmul(wps[:], ident, ident, start=True, stop=True)
            nc.vector.tensor_copy(score_sb[:, 0:P], wps[:])

        if MM_DTYPE == "bf16":
            v_col_mm = consts.tile([P, UC], BF16, name="vcol_mm", tag="vcol_mm")
            nc.vector.tensor_copy(v_col_mm[:], v_col)
        else:
            v_col_mm = v_col

        qe = [consts.tile([P, TQS], F32, name=f"qe{c}", tag=f"qe{c}")
              for c in range(UC)]
        ve = [consts.tile([P, TV], F32, name=f"ve{c}", tag=f"ve{c}")
              for c in range(UC)]

        with tc.tile_pool(name="ph1_ps", bufs=2, space="PSUM") as ph1_ps:
            for c in range(UC):
                ps2 = ph1_ps.tile([P, TV], F32, name="ve_ps", tag="ve_ps")
                for k in range(4):
                    nc.tensor.matmul(ps2[:],
                                     w2_sb[:, k * U + c * P:k * U + (c + 1) * P],
                                     vT[:, k * TV:(k + 1) * TV],
                                     start=(k == 0), stop=(k == 3))
                nc.vector.tensor_copy(ve[c][:], ps2[:])
            for c in range(UC):
                ps = ph1_ps.tile([P, TQS], F32, name="qe_ps", tag="qe_ps")
                for k in range(4):
                    nc.tensor.matmul(ps[:],
                                     w1_sb[:, k * U + c * P:k * U + (c + 1) * P],
                                     qT[:, k * TQS:(k + 1) * TQS],
                                     start=(k == 0), stop=False)
                nc.tensor.matmul(ps[:], b12[:, c * P:(c + 1) * P], ones2,
                                 start=False, stop=True)
                nc.vector.tensor_copy(qe[c][:], ps[:])

        # ---- main loop ----
        n_grouped = G - DIRECT_Q
        escore = work.tile([P, TV], F32, name="escore", tag="escore")
        rowsum = work.tile([P, 1], F32, name="rowsum", tag="rowsum")
        rinv = work.tile([P, 1], F32, name="rinv", tag="rinv")
        attn_sb = work.tile([P, TV], F32, name="attn", tag="attn")
        ctx_sb = work.tile([P, D], F32, name="ctx_sb", tag="ctx_sb")
        stage_k = 0
        with tc.tile_pool(name="score_ps", bufs=2, space="PSUM") as score_ps_pool, \
             tc.tile_pool(name="tail_ps", bufs=2, space="PSUM") as tail_ps, \
             tc.tile_pool(name="ctx_ps_pool", bufs=1, space="PSUM") as ctx_ps_pool:
            ctx_ps = ctx_ps_pool.tile([P, D], F32, name="ctx_ps", tag="ctx_ps")
            sps = None
            for g in range(NG):
                s_t = [s_pool.tile([P, n_grouped * TV], F32, name=f"s{c}",
                                   tag=f"s{c}") for c in range(UC)]
                h_t = [s_pool.tile([P, G * TV], h_dt, name=f"h{c}",
                                   tag=f"h{c}") for c in range(UC)]
                for i in range(n_grouped):
                    q = g * G + i
                    for c in range(UC):
                        if i < GPS_Q:
                            nc.gpsimd.tensor_tensor(
                                out=s_t[c][:, i * TV:(i + 1) * TV],
                                in0=ve[c][:],
                                in1=qe[c][:, q:q + 1].broadcast_to([P, TV]),
                                op=mybir.AluOpType.add)
                        else:
                            nc.vector.tensor_scalar_add(
                                s_t[c][:, i * TV:(i + 1) * TV], ve[c][:],
                                qe[c][:, q:q + 1])
                for c in range(UC):
                    nc.scalar.activation(h_t[c][:, 0:n_grouped * TV], s_t[c][:],
                                         mybir.ActivationFunctionType.Tanh)
                for i in range(n_grouped, G):
                    q = g * G + i
                    for c in range(UC):
                        nc.scalar.activation(h_t[c][:, i * TV:(i + 1) * TV],
                                             ve[c][:],
                                             mybir.ActivationFunctionType.Tanh,
                                             bias=qe[c][:, q:q + 1])
                if g % GPT == 0:
                    sps = score_ps_pool.tile([P, QPT * 64], F32, name="sps",
                                             tag="sps")
                for c in range(UC):          # all chunk-0 passes, then chunk-1
                    for i in range(0, G, 2):
                        w = (g % GPT) * G + i
                        pos, slot = 32 * (w // (QPT // 4)), w % (QPT // 4)
                        nc.tensor.matmul(
                            sps[pos:pos + 1, slot * TV:(slot + 2) * TV],
                            mm_ap(v_col_mm[:, c:c + 1]),
                            mm_ap(h_t[c][:, i * TV:(i + 2) * TV]),
                            start=(c == 0), stop=(c == UC - 1),
                            tile_position=(0, pos))
                if g % GPT == GPT - 1:
                    # PSUM -> SBUF (full-tile copy), then SBUF -> SBUF DMA
                    # regathers rows {0,32,64,96} x slots into QPT score rows.
                    stg = stage_pool.tile([P, QPT * 64], F32, name="stg",
                                          tag="stg")
                    if stage_k % 4 < STAGE_ON_ACT:
                        nc.scalar.copy(stg[:], sps[:])
                    else:
                        nc.vector.tensor_copy(stg[:], sps[:])
                    stage_k += 1
                    t0 = (g - GPT + 1) * G
                    src = stg[:].rearrange("(a b) f -> a b f", b=32)[:, 0, :]
                    nc.sync.dma_start(score_sb[t0:t0 + QPT, :], src)

                    if (t0 + QPT) % 32 != 0:
                        continue
                    # ---- pipelined softmax for rows [r0, r0+32) ----
                    r0 = t0 + QPT - 32
                    sl = slice(r0, r0 + 32)
                    nc.scalar.activation(escore[sl, :], score_sb[sl, :],
                                         mybir.ActivationFunctionType.Exp,
                                         accum_out=rowsum[sl, :])
                    nc.vector.reciprocal(rinv[sl, :], rowsum[sl, :])
                    nc.vector.tensor_scalar_mul(attn_sb[sl, :], escore[sl, :],
                                                rinv[sl, :])
                    nc.sync.dma_start(attn_out[sl, :], attn_sb[sl, :])

            # ---- context = attn @ values ----
            for a in range(2):
                ps = tail_ps.tile([P, P], F32, name="tp2", tag="tp2")
                nc.tensor.transpose(ps[:], attn_sb[:, a * P:(a + 1) * P], ident)
                attnT = work.tile([P, P], F32, name="attnT", tag=f"attnT{a}")
                nc.vector.tensor_copy(attnT[:], ps[:])
                nc.tensor.matmul(ctx_ps[:], attnT[:], values_sb[a],
                                 start=(a == 0), stop=(a == 1))
            nc.vector.tensor_copy(ctx_sb[:], ctx_ps[:])
            nc.sync.dma_start(ctx_out[:], ctx_sb[:])

    nc.finalize()
    return nc


def _get_program():
    global _PROGRAM
    if _PROGRAM is None:
        _PROGRAM = _build_program()
    return _PROGRAM


TRACE = False
RUN_KWARGS = {}
LAST_RESULT = None


def _make_wpack(W1, W2, b1, b2, vv):
    wp = np.zeros((P, _WPACK_COLS), dtype=np.float32)
    wp[:, _IDENT_O:_IDENT_O + P] = np.eye(P, dtype=np.float32)
    wp[0:2, _ONES_O:_ONES_O + P] = 1.0
    wp[0, _B12_O:_B12_O + U] = b1
    wp[1, _B12_O:_B12_O + U] = b2
    for c in range(UC):
        wp[:, _VCOL_O + c] = vv[c * P:(c + 1) * P]
    for k in range(4):
        wp[:, _W1_O + k * U:_W1_O + (k + 1) * U] = W1[k * P:(k + 1) * P, :]
        wp[:, _W2_O + k * U:_W2_O + (k + 1) * U] = W2[k * P:(k + 1) * P, :]
    return wp


def _chunk_cols(x_t):
    """[4P, N] transposed input -> [P, (4 N)] with partition chunks packed
    along columns (chunk k of the contraction dim at cols [k*N, (k+1)*N))."""
    return np.ascontiguousarray(
        x_t.reshape(4, P, x_t.shape[1]).transpose(1, 0, 2).reshape(P, -1))


def kernel(query, values, W1, b1, W2, b2, V, bV):
    global LAST_RESULT
    query = np.asarray(query, dtype=np.float32)
    values = np.asarray(values, dtype=np.float32)
    vv = np.asarray(V, dtype=np.float32).reshape(U)
    wpack = _make_wpack(np.asarray(W1, dtype=np.float32),
                        np.asarray(W2, dtype=np.float32),
                        np.asarray(b1, dtype=np.float32),
                        np.asarray(b2, dtype=np.float32), vv)
    # bV shifts every score equally; softmax is shift-invariant and score is
    # not returned, so it has no effect on either output.

    nc = _get_program()
    in_maps = []
    for core in range(N_CORES):
        b, half = divmod(core, 2)
        qs = query[b, half * TQS:(half + 1) * TQS, :]        # [TQS, H]
        in_maps.append({
            "qT": _chunk_cols(qs.T),                         # [P, 4*TQS]
            "vT": _chunk_cols(values[b].T),                  # [P, 4*TV]
            "v": np.ascontiguousarray(values[b]),
            "wpack": wpack,
        })

    res = run_bass_kernel_spmd(nc, in_maps, list(range(N_CORES)), trace=TRACE,
                               **RUN_KWARGS)
    LAST_RESULT = res

    context = np.empty((B, TQ, D), dtype=np.float32)
    attn = np.empty((B, TQ, TV, 1), dtype=np.float32)
    for core in range(N_CORES):
        b, half = divmod(core, 2)
        sl = slice(half * TQS, (half + 1) * TQS)
        context[b, sl, :] = res.results[core]["ctx"]
        attn[b, sl, :, 0] = res.results[core]["attn"]
    return context, attn


# revision 14
# speedup vs baseline: 1.0337x; 1.0337x over previous
"""Bahdanau attention TRN2 kernel.

Reference math (per batch b):
    qe = query @ W1 + b1                       # [Tq, U]
    ve = values @ W2 + b2                      # [Tv, U]
    score[q, v] = sum_u V[u] * tanh(qe[q, u] + ve[v, u])   (+ bV, dropped:
                  softmax over v is shift-invariant and score is not an
                  output, so bV cancels exactly)
    attn = softmax(score, axis=v)
    context = attn @ values

Sharding: 8 cores = 4 batches x 2 halves of Tq; softmax over Tv is local.

Per-core dataflow (U=256 on partitions as 2 chunks of 128):
  - Host ships layout-transformed inputs: query^T and values^T (so the
    contraction dim is on partitions -- no on-device transposes), plus a
    packed constants array (identity, ones, b1/b2, V columns, W1, W2).
  - PE projects qe_T[u, q] and ve_T[u, v]; b1+b2 folded into qe_T via a
    K=2 ones-matmul (fp32).  A few identity matmuls warm the PE clock
    (HAM) while input DMAs are in flight.
  - Broadcast-add + tanh, engine-balanced per measured rates (DVE
    tensor_scalar 263ns/op, GPSIMD tensor_tensor ~870ns/op, ACT grouped
    tanh ~230ns/q-chunk, ACT direct fused bias+tanh 491ns/op).  tanh
    output H is MM_DTYPE (bf16) for the PE reduction; score still
    accumulates in fp32 PSUM.
  - V-reduction on PE: [K=128, M=1, N=512] matmuls (2 queries each),
    accumulated over the 2 U-chunks (chunk-0 passes emitted before
    chunk-1 so independent matmuls pipeline).  tile_position col-tiling
    spreads rows over PSUM partitions {0,32,64,96} x 4 free slots of a
    2-bank tile (16 q's); a full-tile copy (ACT/DVE alternating) moves it
    to SBUF and an SBUF->SBUF DMA regathers score[q, v] rows.
  - Softmax pipelined per 32-query slice (ACT Exp + fused accum_out
    row-sum, DVE reciprocal + tensor_scalar mul); context at the end via
    PE transpose of attn and two accumulating matmuls against values.
"""

from contextlib import ExitStack

import numpy as np

import concourse.tile as tile
from concourse import bacc, mybir
from concourse.bass_utils import run_bass_kernel_spmd

F32 = mybir.dt.float32
BF16 = mybir.dt.bfloat16

N_CORES = 8
B, TQ, TV = 4, 256, 256
H, D, U = 512, 512, 256
TQS = TQ // 2          # 128 queries per core
P = 128
UC = U // P            # 2 partition chunks of U
G = 16                 # queries per tanh group
NG = TQS // G
QPT = 16               # queries per PSUM score tile (2 banks)
GPT = QPT // G         # groups per score tile

# engine-balance tunables
DIRECT_Q = 1           # q's per group routed via ACT fused bias+tanh
GPS_Q = 0              # q's per group staged by GPSIMD tensor_tensor
STAGE_ON_ACT = 2       # of every 4 psum->sbuf score copies, how many on ACT
WARMUP_MM = 10         # identity matmuls to warm the PE clock
MM_DTYPE = "bf16"      # V-reduction dtype: bf16 | f32 | f32r

# wpack column layout (host-side packing of constants)
_IDENT_O = 0
_VCOL_O = _IDENT_O + P
_W2_O = _VCOL_O + UC          # W2 early: it gates the ve projection
_ONES_O = _W2_O + 4 * U       # rows 0-1 used
_B12_O = _ONES_O + P          # rows 0-1 used: b1 | b2
_W1_O = _B12_O + U
_WPACK_COLS = _W1_O + 4 * U
_WSPLIT = _ONES_O             # DMA 1: ident..W2, DMA 2: ones..W1

_PROGRAM = None


def _build_program():
    nc = bacc.Bacc("TRN2", target_bir_lowering=False, debug=False,
                   num_devices=N_CORES)

    qt_in = nc.declare_dram_parameter("qT", [P, 4 * TQS], F32, isOutput=False)
    vt_in = nc.declare_dram_parameter("vT", [P, 4 * TV], F32, isOutput=False)
    v_in = nc.declare_dram_parameter("v", [TV, D], F32, isOutput=False)
    wp_in = nc.declare_dram_parameter("wpack", [P, _WPACK_COLS], F32,
                                      isOutput=False)
    ctx_out = nc.declare_dram_parameter("ctx", [TQS, D], F32, isOutput=True)
    attn_out = nc.declare_dram_parameter("attn", [TQS, TV], F32, isOutput=True)

    h_dt = {"bf16": BF16, "f32": F32, "f32r": F32}[MM_DTYPE]

    def mm_ap(ap):
        return ap.bitcast(mybir.dt.float32r) if MM_DTYPE == "f32r" else ap

    with tile.TileContext(nc) as tc, ExitStack() as octx:
        consts = octx.enter_context(tc.tile_pool(name="consts", bufs=1))
        work = octx.enter_context(tc.tile_pool(name="work", bufs=1))
        s_pool = octx.enter_context(tc.tile_pool(name="s", bufs=3))
        stage_pool = octx.enter_context(tc.tile_pool(name="stage", bufs=2))

        score_sb = work.tile([P, TV], F32, name="score", tag="score")

        # ---- inputs ----
        wp = consts.tile([P, _WPACK_COLS], F32, name="wp", tag="wp")
        nc.sync.dma_start(wp[:, 0:_WSPLIT], wp_in[:, 0:_WSPLIT])
        ident = wp[:, _IDENT_O:_IDENT_O + P]
        v_col = wp[:, _VCOL_O:_VCOL_O + UC]
        w2_sb = wp[:, _W2_O:_W2_O + 4 * U]

        vT = consts.tile([P, 4 * TV], F32, name="vT", tag="vT")
        nc.sync.dma_start(vT[:], vt_in[:])
        qT = consts.tile([P, 4 * TQS], F32, name="qT", tag="qT")
        nc.sync.dma_start(qT[:], qt_in[:])

        nc.sync.dma_start(wp[:, _WSPLIT:], wp_in[:, _WSPLIT:])
        ones2 = wp[0:2, _ONES_O:_ONES_O + P]
        b12 = wp[0:2, _B12_O:_B12_O + U]
        w1_sb = wp[:, _W1_O:_W1_O + 4 * U]

        values_big = consts.tile([P, 2 * D], F32, name="values", tag="values")
        nc.sync.dma_start(values_big[:].rearrange("p (a d) -> p a d", a=2),
                          v_in[:].rearrange("(a p) d -> p a d", p=P))
        values_sb = [values_big[:, a * D:(a + 1) * D] for a in range(2)]

        # throwaway matmuls warm the PE clock (HAM) during the DMA wait;
        # result parked in score_sb (fully overwritten later)
        with tc.tile_pool(name="warm_ps", bufs=1, space="PSUM") as warm_ps:
            wps = warm_ps.tile([P, P], F32, name="wps", tag="wps")
            for _ in range(WARMUP_MM):
                nc.tensor.matmul(wps[:], ident, ident, start=True, stop=True)
            nc.vector.tensor_copy(score_sb[:, 0:P], wps[:])

        if MM_DTYPE == "bf16":
            v_col_mm = consts.tile([P, UC], BF16, name="vcol_mm", tag="vcol_mm")
            nc.vector.tensor_copy(v_col_mm[:], v_col)
        else:
            v_col_mm = v_col

        qe = [consts.tile([P, TQS], F32, name=f"qe{c}", tag=f"qe{c}")
              for c in range(UC)]
        ve = [consts.tile([P, TV], F32, name=f"ve{c}", tag=f"ve{c}")
              for c in range(UC)]

        with tc.tile_pool(name="ph1_ps", bufs=2, space="PSUM") as ph1_ps:
            for c in range(UC):
                ps2 = ph1_ps.tile([P, TV], F32, name="ve_ps", tag="ve_ps")
                for k in range(4):
                    nc.tensor.matmul(ps2[:],
                                     w2_sb[:, k * U + c * P:k * U + (c + 1) * P],
                                     vT[:, k * TV:(k + 1) * TV],
                                     start=(k == 0), stop=(k == 3))
                nc.vector.tensor_copy(ve[c][:], ps2[:])
            for c in range(UC):
                ps = ph1_ps.tile([P, TQS], F32, name="qe_ps", tag="qe_ps")
                for k in range(4):
                    nc.tensor.matmul(ps[:],
                                     w1_sb[:, k * U + c * P:k * U + (c + 1) * P],
                                     qT[:, k * TQS:(k + 1) * TQS],
                                     start=(k == 0), stop=False)
                nc.tensor.matmul(ps[:], b12[:, c * P:(c + 1) * P], ones2,
                                 start=False, stop=True)
                nc.vector.tensor_copy(qe[c][:], ps[:])

        # ---- main loop ----
        n_grouped = G - DIRECT_Q
        escore = work.tile([P, TV], F32, name="escore", tag="escore")
        rowsum = work.tile([P, 1], F32, name="rowsum", tag="rowsum")
        rinv = work.tile([P, 1], F32, name="rinv", tag="rinv")
        attn_sb = work.tile([P, TV], F32, name="attn", tag="attn")
        ctx_sb = work.tile([P, D], F32, name="ctx_sb", tag="ctx_sb")
        stage_k = 0
        with tc.tile_pool(name="score_ps", bufs=2, space="PSUM") as score_ps_pool, \
             tc.tile_pool(name="tail_ps", bufs=2, space="PSUM") as tail_ps, \
             tc.tile_pool(name="ctx_ps_pool", bufs=1, space="PSUM") as ctx_ps_pool:
            ctx_ps = ctx_ps_pool.tile([P, D], F32, name="ctx_ps", tag="ctx_ps")
            sps = None
            for g in range(NG):
                s_t = [s_pool.tile([P, n_grouped * TV], F32, name=f"s{c}",
                                   tag=f"s{c}") for c in range(UC)]
                h_t = [s_pool.tile([P, G * TV], h_dt, name=f"h{c}",
                                   tag=f"h{c}") for c in range(UC)]
                for i in range(n_grouped):
                    q = g * G + i
                    for c in range(UC):
                        if i < GPS_Q:
                            nc.gpsimd.tensor_tensor(
                                out=s_t[c][:, i * TV:(i + 1) * TV],
                                in0=ve[c][:],
                                in1=qe[c][:, q:q + 1].broadcast_to([P, TV]),
                                op=mybir.AluOpType.add)
                        else:
                            nc.vector.tensor_scalar_add(
                                s_t[c][:, i * TV:(i + 1) * TV], ve[c][:],
                                qe[c][:, q:q + 1])
                for c in range(UC):
                    nc.scalar.activation(h_t[c][:, 0:n_grouped * TV], s_t[c][:],
                                         mybir.ActivationFunctionType.Tanh)
                for i in range(n_grouped, G):
                    q = g * G + i
                    for c in range(UC):
                        nc.scalar.activation(h_t[c][:, i * TV:(i + 1) * TV],
                                             ve[c][:],
                                             mybir.ActivationFunctionType.Tanh,
                                             bias=qe[c][:, q:q + 1])
                if g % GPT == 0:
                    sps = score_ps_pool.tile([P, QPT * 64], F32, name="sps",
                                             tag="sps")
                for c in range(UC):          # all chunk-0 passes, then chunk-1
                    for i in range(0, G, 2):
                        w = (g % GPT) * G + i
                        pos, slot = 32 * (w // (QPT // 4)), w % (QPT // 4)
                        nc.tensor.matmul(
                            sps[pos:pos + 1, slot * TV:(slot + 2) * TV],
                            mm_ap(v_col_mm[:, c:c + 1]),
                            mm_ap(h_t[c][:, i * TV:(i + 2) * TV]),
                            start=(c == 0), stop=(c == UC - 1),
                            tile_position=(0, pos))
                if g % GPT == GPT - 1:
                    # PSUM -> SBUF (full-tile copy), then SBUF -> SBUF DMA
                    # regathers rows {0,32,64,96} x slots into QPT score rows.
                    stg = stage_pool.tile([P, QPT * 64], F32, name="stg",
                                          tag="stg")
                    if stage_k % 4 < STAGE_ON_ACT:
                        nc.scalar.copy(stg[:], sps[:])
                    else:
                        nc.vector.tensor_copy(stg[:], sps[:])
                    stage_k += 1
                    t0 = (g - GPT + 1) * G
                    src = stg[:].rearrange("(a b) f -> a b f", b=32)[:, 0, :]
                    nc.sync.dma_start(score_sb[t0:t0 + QPT, :], src)

                    if (t0 + QPT) % 32 != 0:
                        continue
                    # ---- pipelined softmax for rows [r0, r0+32) ----
                    r0 = t0 + QPT - 32
                    sl = slice(r0, r0 + 32)
                    nc.scalar.activation(escore[sl, :], score_sb[sl, :],
                                         mybir.ActivationFunctionType.Exp,
                                         accum_out=rowsum[sl, :])
                    nc.vector.reciprocal(rinv[sl, :], rowsum[sl, :])
                    nc.vector.tensor_scalar_mul(attn_sb[sl, :], escore[sl, :],
                                                rinv[sl, :])
                    nc.sync.dma_start(attn_out[sl, :], attn_sb[sl, :])

            # ---- context = attn @ values ----
            for a in range(2):
                ps = tail_ps.tile([P, P], F32, name="tp2", tag="tp2")
                nc.tensor.transpose(ps[:], attn_sb[:, a * P:(a + 1) * P], ident)
                attnT = work.tile([P, P], F32, name="attnT", tag=f"attnT{a}")
                nc.vector.tensor_copy(attnT[:], ps[:])
                nc.tensor.matmul(ctx_ps[:], attnT[:], values_sb[a],
                                 start=(a == 0), stop=(a == 1))
            nc.vector.tensor_copy(ctx_sb[:], ctx_ps[:])
            nc.sync.dma_start(ctx_out[:], ctx_sb[:])

    nc.finalize()
    return nc


def _get_program():
    global _PROGRAM
    if _PROGRAM is None:
        _PROGRAM = _build_program()
    return _PROGRAM


TRACE = False
RUN_KWARGS = {}
LAST_RESULT = None


def _make_wpack(W1, W2, b1, b2, vv):
    wp = np.zeros((P, _WPACK_COLS), dtype=np.float32)
    wp[:, _IDENT_O:_IDENT_O + P] = np.eye(P, dtype=np.float32)
    wp[0:2, _ONES_O:_ONES_O + P] = 1.0
    wp[0, _B12_O:_B12_O + U] = b1
    wp[1, _B12_O:_B12_O + U] = b2
    for c in range(UC):
        wp[:, _VCOL_O + c] = vv[c * P:(c + 1) * P]
    for k in range(4):
        wp[:, _W1_O + k * U:_W1_O + (k + 1) * U] = W1[k * P:(k + 1) * P, :]
        wp[:, _W2_O + k * U:_W2_O + (k + 1) * U] = W2[k * P:(k + 1) * P, :]
    return wp


def _chunk_cols(x_t):
    """[4P, N] transposed input -> [P, (4 N)] with partition chunks packed
    along columns (chunk k of the contraction dim at cols [k*N, (k+1)*N))."""
    return np.ascontiguousarray(
        x_t.reshape(4, P, x_t.shape[1]).transpose(1, 0, 2).reshape(P, -1))


def kernel(query, values, W1, b1, W2, b2, V, bV):
    global LAST_RESULT
    query = np.asarray(query, dtype=np.float32)
    values = np.asarray(values, dtype=np.float32)
    vv = np.asarray(V, dtype=np.float32).reshape(U)
    wpack = _make_wpack(np.asarray(W1, dtype=np.float32),
                        np.asarray(W2, dtype=np.float32),
                        np.asarray(b1, dtype=np.float32),
                        np.asarray(b2, dtype=np.float32), vv)
    # bV shifts every score equally; softmax is shift-invariant and score is
    # not returned, so it has no effect on either output.

    nc = _get_program()
    in_maps = []
    for core in range(N_CORES):
        b, half = divmod(core, 2)
        qs = query[b, half * TQS:(half + 1) * TQS, :]        # [TQS, H]
        in_maps.append({
            "qT": _chunk_cols(qs.T),                         # [P, 4*TQS]
            "vT": _chunk_cols(values[b].T),                  # [P, 4*TV]
            "v": np.ascontiguousarray(values[b]),
            "wpack": wpack,
        })

    res = run_bass_kernel_spmd(nc, in_maps, list(range(N_CORES)), trace=TRACE,
                               **RUN_KWARGS)
    LAST_RESULT = res

    context = np.empty((B, TQ, D), dtype=np.float32)
    attn = np.empty((B, TQ, TV, 1), dtype=np.float32)
    for core in range(N_CORES):
        b, half = divmod(core, 2)
        sl = slice(half * TQS, (half + 1) * TQS)
        context[b, sl, :] = res.results[core]["ctx"]
        attn[b, sl, :, 0] = res.results[core]["attn"]
    return context, attn
